# revision 1
# baseline (speedup 1.0000x reference)
"""Trainium2 Bass kernel for the ClassificationNCA problem.

Self-contained: callable as kernel(**inputs) with the full (unsharded)
inputs; shards batch across 8 NeuronCores (2 images/core), runs the
20-step NCA entirely in SBUF, returns softmax(mean-pooled class channels).

Layout (per core): state lives in SBUF as [128, 34, 66] fp32 where
partition = img*64 + half*32 + ch (29 channels + 3 pad partitions per
block; each (img, half) block holds a 32-row half-image strip with one
halo row on the inner edge and a zero pad row on the outer edge; columns
are zero-padded to 66).  This packs the sobel stencil work onto all 128
partitions and puts every matmul operand at a 32-aligned base partition.
"""
import sys

sys.path.insert(0, "/opt/trn_rl_repo")

import json
import numpy as np
import ml_dtypes

NUM_IMG, NUM_HID, NUM_OUT = 3, 16, 10
NCH = 29            # total channels
HIDDEN = 128
B, H, W = 16, 64, 64
N_CORES = 8
B_LOC = B // N_CORES          # images per core
WP = W + 4                    # padded width (68): 2 pad cols each side keeps
                              # interior rows 4B-aligned in bf16 for DVE 4x copies
SR = 34                       # strip rows: pad/halo + 32 interior + halo/pad
SFLAT = SR * WP               # 2312
ROWS_PER_TILE = 8
N_RB = 32 // ROWS_PER_TILE    # 4 row blocks per (img, half) strip
NPIX_TILE = ROWS_PER_TILE * W  # 512
CB = 32                       # channel-block partition stride
CL = 2                        # interior column offset

_MAX_WAITS = 1


def _fix_bir_waits(bir_bytes: bytes) -> bytes:
    """walrus codegen allows only one embedded sem-wait per instruction;
    Tile sometimes attaches more (e.g. the kernel-tail drain).  Move excess
    waits onto NoOp carrier instructions inserted before the offender on the
    same engine."""
    bir = json.loads(bir_bytes)
    uid = 0
    for fn in bir["functions"]:
        for blk in fn["blocks"]:
            out = []
            for ins in blk["instructions"]:
                si = ins.get("sync_info")
                waits = (si or {}).get("on_wait") or []
                if len(waits) > _MAX_WAITS:
                    excess = waits[:-_MAX_WAITS]
                    si["on_wait"] = waits[-_MAX_WAITS:]
                    for i in range(0, len(excess), _MAX_WAITS):
                        out.append({
                            "opcode": "NoOp",
                            "name": f"wsplit_{uid}",
                            "engine": ins["engine"],
                            "ins": [],
                            "outs": [],
                            "sync_info": {
                                "on_wait": excess[i:i + _MAX_WAITS],
                                "on_update": [],
                            },
                        })
                        uid += 1
                out.append(ins)
            blk["instructions"] = out
    return json.dumps(bir).encode()


def _host_rng(steps: int):
    """Reproduce the reference's jax threefry random draws exactly (on CPU)."""
    import jax
    cpu = jax.devices("cpu")[0]
    with jax.default_device(cpu):
        import jax.numpy as jnp
        base = jax.random.key(42)
        hid = 0.5 + 0.225 * jax.random.normal(
            jax.random.fold_in(base, 10_000), (B, NUM_HID, H, W),
            dtype=jnp.float32)
        hid = np.asarray(hid)
        fires = np.zeros((max(steps, 1), B, H, W), np.float32)
        for s in range(steps):
            u = jax.random.uniform(jax.random.fold_in(base, s), (B, H, W, 1),
                                   dtype=jnp.float32)
            fires[s] = np.asarray(u < 0.5, np.float32)[..., 0]
    return hid, fires


def _build(steps: int, repeat: int = 1):
    from concourse import mybir
    from concourse.bass import Bass
    from concourse.tile import TileContext

    f32 = mybir.dt.float32
    bf16 = mybir.dt.bfloat16
    LR = mybir.ActivationFunctionType.Lrelu

    nc = Bass(trn_type="TRN2", target_bir_lowering=False)

    s0_d = nc.dram_tensor("s0", [128, SR, WP], f32, kind="ExternalInput")
    fire_d = nc.dram_tensor("fire", [max(steps, 1), 128, B_LOC, H, W],
                            bf16, kind="ExternalInput")
    # L1 weights: s-tap weights quadruplicated at bases 0/32/64/96 (the rhs
    # comes straight from the bf16 state shadow at the block's partitions);
    # sx/sy weights concatenated for the K=64 gathered tap.
    w1sq_d = nc.dram_tensor("w1sq", [96 + NCH, 2 * HIDDEN], bf16, kind="ExternalInput")
    w1xy_d = nc.dram_tensor("w1xy", [2 * CB, 2 * HIDDEN], bf16, kind="ExternalInput")
    w2T_d = nc.dram_tensor("w2T", [HIDDEN, 2 * HIDDEN], bf16, kind="ExternalInput")
    w3T_d = nc.dram_tensor("w3T", [HIDDEN, NCH], bf16, kind="ExternalInput")
    b1e_d = nc.dram_tensor("b1e", [HIDDEN, max(2 * steps, 1)], f32, kind="ExternalInput")
    out_d = nc.dram_tensor("sout", [128, SR, WP], f32, kind="ExternalOutput")

    with TileContext(nc) as tc:
        with tc.tile_pool(name="state", bufs=1) as statep, \
             tc.tile_pool(name="wts", bufs=1) as wtsp, \
             tc.tile_pool(name="convs", bufs=1) as convp, \
             tc.tile_pool(name="fire", bufs=2) as firep, \
             tc.tile_pool(name="acts", bufs=6) as actp, \
             tc.tile_pool(name="perc", bufs=20) as percp, \
             tc.tile_pool(name="ph1", bufs=2, space="PSUM") as ph1, \
             tc.tile_pool(name="ph2", bufs=2, space="PSUM") as ph2, \
             tc.tile_pool(name="pdx", bufs=1, space="PSUM") as pdx:

            S = [statep.tile([128, SR, WP], f32, name=f"S{k}") for k in range(2)]
            S16 = [statep.tile([128, SR, WP], bf16, name=f"S16_{k}") for k in range(2)]
            SX = [statep.tile([128, SR, WP], bf16, name=f"SX{k}") for k in range(2)]
            SY = [statep.tile([128, SR, WP], bf16, name=f"SY{k}") for k in range(2)]
            T1 = convp.tile([128, SFLAT], bf16, name="T1")
            T2 = convp.tile([128, SFLAT], bf16, name="T2")
            T3 = convp.tile([128, SFLAT], bf16, name="T3")
            T4 = convp.tile([128, SFLAT], bf16, name="T4")

            w1sq = wtsp.tile([96 + NCH, 2 * HIDDEN], bf16, name="w1sq")
            w1xy = wtsp.tile([2 * CB, 2 * HIDDEN], bf16, name="w1xy")
            w2T = wtsp.tile([HIDDEN, 2 * HIDDEN], bf16, name="w2T")
            w3T = wtsp.tile([HIDDEN, NCH], bf16, name="w3T")
            b1e = wtsp.tile([HIDDEN, max(2 * steps, 1)], f32, name="b1e")

            nc.sync.dma_start(out=S[0][:], in_=s0_d[:])
            nc.sync.dma_start(out=S[1][:], in_=s0_d[:])
            nc.sync.dma_start(out=w1sq[:], in_=w1sq_d[:])
            nc.sync.dma_start(out=w1xy[:], in_=w1xy_d[:])
            nc.sync.dma_start(out=w2T[:], in_=w2T_d[:])
            nc.sync.dma_start(out=w3T[:], in_=w3T_d[:])
            nc.sync.dma_start(out=b1e[:], in_=b1e_d[:])

            def emit_conv(g, i):
                """Sobel pass + bf16 shadow for step g, image i (reads S[g%2],
                which must already carry step g's state for image i).  Pool
                ops and DVE ops are emitted directly."""
                cur = S[g % 2]
                s16, sx, sy = S16[g % 2], SX[g % 2], SY[g % 2]
                ip = i * 2 * CB
                pr = slice(ip, ip + 2 * CB)
                cf = cur[pr, :, :].rearrange("p a b -> p (a b)")
                sxf = sx[pr, :, :].rearrange("p a b -> p (a b)")
                syf = sy[pr, :, :].rearrange("p a b -> p (a b)")

                # bf16 shadow of this image's state (pool engine), then
                # sobel-y horizontal smooth (B then HS) also on pool
                nc.gpsimd.tensor_copy(out=s16[pr], in_=cur[pr])
                s16f = s16[pr, :, :].rearrange("p a b -> p (a b)")
                nc.vector.tensor_add(out=T3[pr, 0:SFLAT - 1],
                                     in0=s16f[:, 0:SFLAT - 1],
                                     in1=s16f[:, 1:SFLAT])
                nc.gpsimd.tensor_add(out=T4[pr, 1:SFLAT - 1],
                                     in0=T3[pr, 0:SFLAT - 2],
                                     in1=T3[pr, 1:SFLAT - 1])

                # sobel-x vertical smooth (A then VS, bf16 2x), horizontal
                # diff, and sobel-y vertical diff -- on DVE
                nc.vector.tensor_add(out=T1[pr, 0:SFLAT - WP],
                                     in0=s16f[:, 0:SFLAT - WP],
                                     in1=s16f[:, WP:SFLAT])
                nc.vector.tensor_add(out=T2[pr, WP:SFLAT - WP],
                                     in0=T1[pr, 0:SFLAT - 2 * WP],
                                     in1=T1[pr, WP:SFLAT - WP])
                nc.vector.tensor_sub(out=sxf[:, WP + 1:SFLAT - WP - 1],
                                     in0=T2[pr, WP + 2:SFLAT - WP],
                                     in1=T2[pr, WP:SFLAT - WP - 2])
                nc.vector.tensor_sub(out=syf[:, WP + 1:SFLAT - WP - 1],
                                     in0=T4[pr, 2 * WP + 1:SFLAT - 1],
                                     in1=T4[pr, 1:SFLAT - 2 * WP - 1])

            def emit_copies(g, i):
                """Per-tile [sx|sy] gathers for step g, image i (the 3 pad
                partitions of each source block are zero)."""
                sx, sy = SX[g % 2], SY[g % 2]
                percs = []
                for hf in range(2):
                    p0 = i * 2 * CB + hf * CB
                    for rb in range(N_RB):
                        r0 = 1 + rb * ROWS_PER_TILE
                        perc = percp.tile([2 * CB, ROWS_PER_TILE, W], bf16,
                                          name="perc")
                        nc.vector.tensor_copy(
                            out=perc[0:CB],
                            in_=sx[p0:p0 + CB, r0:r0 + ROWS_PER_TILE, CL:CL + W])
                        nc.vector.tensor_copy(
                            out=perc[CB:2 * CB],
                            in_=sy[p0:p0 + CB, r0:r0 + ROWS_PER_TILE, CL:CL + W])
                        percs.append(perc)
                return percs

            def emit_wave(g, i, ft, percs):
                t = g % steps
                cur, nxt = S[g % 2], S[(g + 1) % 2]
                s16, sy = S16[g % 2], SY[g % 2]
                ip = i * 2 * CB
                for hf in range(2):
                    p0 = i * 2 * CB + hf * CB
                    dxp = None
                    for rb in range(N_RB):
                        if rb % 2 == 0:
                            dxp = pdx.tile([NCH, 2 * ROWS_PER_TILE, W], f32,
                                           name="dx")
                        r0 = 1 + rb * ROWS_PER_TILE
                        perc = percs[hf * N_RB + rb]
                        percf = perc[:, :, :].rearrange("p a b -> p (a b)")
                        rhs_s = s16[p0:p0 + NCH, r0:r0 + ROWS_PER_TILE, CL:CL + W]

                        h1 = ph1.tile([HIDDEN, 2 * NPIX_TILE], f32, name="h1")
                        for half in range(2):
                            c0 = half * HIDDEN
                            o = h1[:, half * NPIX_TILE:(half + 1) * NPIX_TILE]
                            nc.tensor.matmul(o, w1sq[p0:p0 + NCH, c0:c0 + HIDDEN],
                                             rhs_s, start=True, stop=False,
                                             tile_position=(p0, 0))
                            nc.tensor.matmul(o, w1xy[:, c0:c0 + HIDDEN],
                                             percf, start=False, stop=True)

                        h1s = actp.tile([HIDDEN, 2 * NPIX_TILE], bf16, name="h1s")
                        for half in range(2):
                            sl = slice(half * NPIX_TILE, (half + 1) * NPIX_TILE)
                            nc.scalar.activation(
                                out=h1s[:, sl], in_=h1[:, sl], func=LR,
                                bias=b1e[:, 2 * t + half:2 * t + half + 1],
                                scale=1.0, alpha=0.01)

                        h2 = ph2.tile([HIDDEN, NPIX_TILE], f32, name="h2")
                        nc.tensor.matmul(h2[:], w2T[:, 0:HIDDEN],
                                         h1s[:, 0:NPIX_TILE], start=True, stop=False)
                        nc.tensor.matmul(h2[:], w2T[:, HIDDEN:2 * HIDDEN],
                                         h1s[:, NPIX_TILE:2 * NPIX_TILE],
                                         start=False, stop=True)

                        h2s = actp.tile([HIDDEN, NPIX_TILE], bf16, name="h2s")
                        nc.scalar.activation(out=h2s[:], in_=h2[:], func=LR,
                                             bias=0.0, scale=1.0, alpha=0.01)

                        # stochastic fire mask applied to h2 (per-pixel,
                        # broadcast over channels) before the last matmul
                        fr0 = hf * 32 + rb * ROWS_PER_TILE
                        h2m = actp.tile([HIDDEN, ROWS_PER_TILE, W], bf16, name="h2m")
                        nc.vector.tensor_mul(
                            out=h2m[:],
                            in0=h2s[:].rearrange("p (a b) -> p a b", a=ROWS_PER_TILE),
                            in1=ft[:, i, fr0:fr0 + ROWS_PER_TILE, :])

                        sub = rb % 2
                        dxf = dxp[:, :, :].rearrange("p a b -> p (a b)")
                        nc.tensor.matmul(
                            dxf[:, sub * NPIX_TILE:(sub + 1) * NPIX_TILE],
                            w3T[:],
                            h2m[:, :, :].rearrange("p a b -> p (a b)"),
                            start=True, stop=True)

                        if sub == 1:
                            r0p = r0 - ROWS_PER_TILE
                            nc.vector.tensor_add(
                                out=nxt[p0:p0 + NCH, r0p:r0p + 2 * ROWS_PER_TILE,
                                        CL:CL + W],
                                in0=cur[p0:p0 + NCH, r0p:r0p + 2 * ROWS_PER_TILE,
                                        CL:CL + W],
                                in1=dxp[:])

                # halo-row sync between this image's two half-strips:
                #   half1.strip[0] <- half0.strip[32];
                #   half0.strip[33] <- half1.strip[1]
                nc.gpsimd.tensor_copy(out=nxt[ip + CB:ip + CB + NCH, 0:1, :],
                                      in_=nxt[ip:ip + NCH, 32:33, :])
                nc.gpsimd.tensor_copy(out=nxt[ip:ip + NCH, 33:34, :],
                                      in_=nxt[ip + CB:ip + CB + NCH, 1:2, :])

            # Software-pipelined emission: the sobel pass and sx-gathers for
            # step g+1 of image i are emitted right after image i's wave of
            # step g, so they fill the other engines while the other image's
            # wave runs.  Per-engine streams execute in order, so this
            # ordering is what creates the overlap.
            TOT = steps * repeat
            fts = {}

            def get_ft(g):
                if g not in fts:
                    ftt = firep.tile([128, B_LOC, H, W], bf16, name="ft")
                    nc.sync.dma_start(out=ftt[:], in_=fire_d[g % steps])
                    fts[g] = ftt
                return fts[g]

            # warm the PE p-state during the init DMAs so step 0's matmuls
            # run at full clock
            for _w in range(24):
                wp = ph1.tile([HIDDEN, 2 * NPIX_TILE], f32, name="h1")
                nc.tensor.matmul(wp[:, 0:HIDDEN], w2T[:, 0:HIDDEN],
                                 w2T[:, 0:HIDDEN], start=True, stop=True)

            for g in range(TOT):
                for i in range(B_LOC):
                    emit_conv(g, i)
                    percs = emit_copies(g, i)
                    emit_wave(g, i, get_ft(g), percs)

            nc.sync.dma_start(out=out_d[:], in_=S[TOT % 2][:])

    orig = nc.to_json_bytes
    nc.to_json_bytes = lambda: _fix_bir_waits(orig())
    return nc


_CACHE = {}


def _get_nc(steps: int, repeat: int = 1):
    key = (steps, repeat)
    if key not in _CACHE:
        _CACHE[key] = _build(steps, repeat)
    return _CACHE[key]


def _prep_inputs(x, w1, b1, w2, w3, steps):
    """Host-side input preparation; returns per-core input maps."""
    x = np.asarray(x, np.float32)
    w1 = np.asarray(w1, np.float32)
    b1 = np.asarray(b1, np.float32)
    w2 = np.asarray(w2, np.float32)
    w3 = np.asarray(w3, np.float32)

    hid, fires = _host_rng(steps)

    # full padded state [B, 32, 66, 68] (channel blocks padded to 32)
    state0 = np.zeros((B, CB, H + 2, WP), np.float32)
    state0[:, :NUM_IMG, 1:1 + H, CL:CL + W] = x
    state0[:, NUM_IMG:NUM_IMG + NUM_HID, 1:1 + H, CL:CL + W] = hid

    bf = ml_dtypes.bfloat16

    w1sq = np.zeros((96 + NCH, 2 * HIDDEN), np.float32)
    for b0 in (0, 32, 64, 96):
        w1sq[b0:b0 + NCH] = w1[:, 0:NCH].T
    w1sq = w1sq.astype(bf)
    w1xy = np.zeros((2 * CB, 2 * HIDDEN), np.float32)
    w1xy[0:NCH] = w1[:, NCH:2 * NCH].T / 8.0
    w1xy[CB:CB + NCH] = w1[:, 2 * NCH:3 * NCH].T / 8.0
    w1xy = w1xy.astype(bf)
    w2T = np.concatenate([w2[:, :HIDDEN].T, w2[:, HIDDEN:].T], axis=1)
    w2T = np.ascontiguousarray(w2T).astype(bf)
    w3Tf = w3.T.copy()           # [128, 29]
    w3Tf[:, :NUM_IMG] = 0.0      # image channels are immutable
    w3T = np.ascontiguousarray(w3Tf).astype(bf)

    nb = max(2 * steps, 1)
    b1e = np.zeros((HIDDEN, nb), np.float32)
    for t in range(steps):
        be = b1 + w1[:, 3 * NCH] * (np.float32(t) / np.float32(100.0))
        b1e[:, 2 * t] = be[0:HIDDEN]
        b1e[:, 2 * t + 1] = be[HIDDEN:2 * HIDDEN]

    in_maps = []
    for c in range(N_CORES):
        imgs = slice(c * B_LOC, (c + 1) * B_LOC)
        sc = state0[imgs]                      # [B_LOC, 32, 66, 66]
        # strips: half0 = rows 0:34, half1 = rows 32:66
        s0 = np.stack([sc[:, :, 0:SR, :], sc[:, :, 32:32 + SR, :]], axis=1)
        s0 = s0.reshape(B_LOC * 2 * CB, SR, WP)
        f = fires[:max(steps, 1), imgs]                      # [steps, B_LOC, H, W]
        f = np.broadcast_to(f[:, None], (max(steps, 1), 128, B_LOC, H, W))
        in_maps.append({
            "s0": np.ascontiguousarray(s0),
            "fire": np.ascontiguousarray(f).astype(bf),
            "w1sq": w1sq, "w1xy": w1xy,
            "w2T": w2T, "w3T": w3T, "b1e": b1e,
        })
    return in_maps


def _softmax(x):
    m = x.max(axis=-1, keepdims=True)
    e = np.exp(x - m)
    return e / e.sum(axis=-1, keepdims=True)


def _epilogue(results):
    logits = np.zeros((B, NUM_OUT), np.float32)
    for c, res in enumerate(results):
        so = res["sout"].reshape(B_LOC, 2, CB, SR, WP)
        cls = so[:, :, NUM_IMG + NUM_HID:NCH, 1:33, CL:CL + W]  # [B_LOC, 2, 10, 32, 64]
        logits[c * B_LOC:(c + 1) * B_LOC] = cls.mean(axis=(1, 3, 4))
    return _softmax(logits).astype(np.float32)


def _run(trace=False, repeat=1, _in_maps=None, **inputs):
    from concourse.bass_utils import run_bass_kernel_spmd
    steps = int(inputs["steps"])
    if steps == 0:
        return _softmax(np.zeros((B, NUM_OUT), np.float32)), None
    in_maps = _in_maps
    if in_maps is None:
        in_maps = _prep_inputs(inputs["x"], inputs["w1"], inputs["b1"],
                               inputs["w2"], inputs["w3"], steps)
    nc = _get_nc(steps, repeat)
    r = run_bass_kernel_spmd(nc, in_maps, core_ids=list(range(N_CORES)),
                             trace=trace)
    return _epilogue(r.results), r.exec_time_ns


def predicted_exec_ns(steps: int = 20) -> float:
    """Cost-model (TimelineSim) estimate of on-device execution time for the
    whole job (all cores run the same program in parallel).  The axon
    container has no NTFF profiling path, so this is the best available
    hardware-time number; it uses the same instruction cost model the
    perfetto tooling is built on."""
    from concourse.timeline_sim import TimelineSim
    nc = _build(int(steps))
    return TimelineSim(nc, trace=False).simulate()


def kernel(**inputs) -> np.ndarray:
    out, _ = _run(trace=False, **inputs)
    return out



# revision 17
# speedup vs baseline: 1.2955x; 1.2955x over previous
"""Trainium2 Bass kernel for the ClassificationNCA problem.

Self-contained: callable as kernel(**inputs) with the full (unsharded)
inputs; shards batch across 8 NeuronCores (2 images/core), runs the
20-step NCA entirely in SBUF, returns softmax(mean-pooled class channels).

Layout (per core): state lives in SBUF as [128, 34, 68] fp32 where
partition = 32*block + ch, block = 2*img + half (29 channels + pad;
partition 32b+29 is held at constant 1.0 and acts as the bias input row).
Each block holds a 32-row half-image strip with halo rows at the strip
edges and columns zero-padded to 68.

Per step: full-width (128-partition) separable sobel in bf16; the fire
mask is folded into the L1 matmul inputs (fired shadow SPerc / fired
sobel gathers XY, with the constant-1.0 row turning into the per-pixel
fire value that gates the folded bias), so nothing downstream of L1
needs a fire multiply; L2 runs as an fp8e4 DoubleRow matmul; state
updates accumulate 4 blocks at once from a shared PSUM dx tile.
"""
import sys

sys.path.insert(0, "/opt/trn_rl_repo")

import json
import numpy as np
import ml_dtypes

NUM_IMG, NUM_HID, NUM_OUT = 3, 16, 10
NCH = 29            # total channels
HIDDEN = 128
B, H, W = 16, 64, 64
N_CORES = 8
B_LOC = B // N_CORES          # images per core
WP = W + 4                    # padded width: 2 pad cols each side
SR = 34                       # strip rows: pad/halo + 32 interior + halo/pad
SFLAT = SR * WP               # 2312
RPT = 8                       # rows per matmul tile
N_RB = 32 // RPT              # 4 row blocks per strip
NPIX = RPT * W                # 512 pixels per tile
CB = 32                       # partition stride per block
CL = 2                        # interior column offset

_MAX_WAITS = 1

# h2 leaky-relu engine per tile emission index: balance Act/Pool/DVE.
# "A": Activation-engine LeakyRelu; "V"/"P": dual-matmul trick -- a second
# DoubleRow pass with 0.01-scaled weights, then tensor_max on DVE/Pool.
_H2_ENG = ["A"] * 16
# wave processes row-blocks in this order so the next step's conv slabs
# (chain {1,2}, then slab 0, slab 3) become ready in consumption order
_WAVE_ORDER = [1, 0, 2, 3]


def _fix_bir_waits(bir_bytes: bytes) -> bytes:
    """walrus codegen allows only one embedded sem-wait per instruction;
    Tile sometimes attaches more (e.g. the kernel-tail drain).  Move excess
    waits onto NoOp carrier instructions inserted before the offender on the
    same engine."""
    bir = json.loads(bir_bytes)
    uid = 0
    for fn in bir["functions"]:
        for blk in fn["blocks"]:
            out = []
            for ins in blk["instructions"]:
                si = ins.get("sync_info")
                waits = (si or {}).get("on_wait") or []
                if len(waits) > _MAX_WAITS:
                    excess = waits[:-_MAX_WAITS]
                    si["on_wait"] = waits[-_MAX_WAITS:]
                    for i in range(0, len(excess), _MAX_WAITS):
                        out.append({
                            "opcode": "NoOp",
                            "name": f"wsplit_{uid}",
                            "engine": ins["engine"],
                            "ins": [],
                            "outs": [],
                            "sync_info": {
                                "on_wait": excess[i:i + _MAX_WAITS],
                                "on_update": [],
                            },
                        })
                        uid += 1
                out.append(ins)
            blk["instructions"] = out
    return json.dumps(bir).encode()


def _host_rng(steps: int):
    """Reproduce the reference's jax threefry random draws exactly (on CPU)."""
    import jax
    cpu = jax.devices("cpu")[0]
    with jax.default_device(cpu):
        import jax.numpy as jnp
        base = jax.random.key(42)
        hid = 0.5 + 0.225 * jax.random.normal(
            jax.random.fold_in(base, 10_000), (B, NUM_HID, H, W),
            dtype=jnp.float32)
        hid = np.asarray(hid)
        fires = np.zeros((max(steps, 1), B, H, W), np.float32)
        for s in range(steps):
            u = jax.random.uniform(jax.random.fold_in(base, s), (B, H, W, 1),
                                   dtype=jnp.float32)
            fires[s] = np.asarray(u < 0.5, np.float32)[..., 0]
    return hid, fires


def _build(steps: int):
    from concourse import mybir
    from concourse.bass import Bass
    from concourse.tile import TileContext

    f32 = mybir.dt.float32
    bf16 = mybir.dt.bfloat16
    fp8 = mybir.dt.float8e4
    fp8e5 = mybir.dt.float8e5
    LR = mybir.ActivationFunctionType.Lrelu
    DR = mybir.MatmulPerfMode.DoubleRow
    MULT = mybir.AluOpType.mult
    MAX = mybir.AluOpType.max

    nst = max(steps, 1)

    nc = Bass(trn_type="TRN2", target_bir_lowering=False)

    s0_d = nc.dram_tensor("s0", [128, SR, WP], f32, kind="ExternalInput")
    fire_d = nc.dram_tensor("fire", [nst, 128, SR, WP], bf16,
                            kind="ExternalInput")
    w1sT_d = nc.dram_tensor("w1sT", [128, nst, 2 * HIDDEN], bf16,
                            kind="ExternalInput")
    w1xy_d = nc.dram_tensor("w1xy", [128, 2, 2 * HIDDEN], bf16,
                            kind="ExternalInput")
    w2dr_d = nc.dram_tensor("w2dr", [HIDDEN, 2, HIDDEN], fp8, kind="ExternalInput")
    w2drb_d = nc.dram_tensor("w2drb", [HIDDEN, 2, HIDDEN], fp8e5, kind="ExternalInput")
    w3T_d = nc.dram_tensor("w3T", [HIDDEN, CB], bf16, kind="ExternalInput")
    out_d = nc.dram_tensor("sout", [128, SR, WP], f32, kind="ExternalOutput")

    with TileContext(nc) as tc:
        with tc.tile_pool(name="state", bufs=1) as statep, \
             tc.tile_pool(name="wts", bufs=1) as wtsp, \
             tc.tile_pool(name="convs", bufs=1) as convp, \
             tc.tile_pool(name="fire", bufs=2) as firep, \
             tc.tile_pool(name="acts", bufs=4) as actp, \
             tc.tile_pool(name="ph1", bufs=2, space="PSUM") as ph1, \
             tc.tile_pool(name="ph2", bufs=2, space="PSUM") as ph2, \
             tc.tile_pool(name="pdx", bufs=2, space="PSUM") as pdx:

            S = [statep.tile([128, SR, WP], f32, name=f"S{k}") for k in range(2)]
            s16 = statep.tile([128, SR, WP], bf16, name="s16")
            SXp = statep.tile([128, 32, W], bf16, name="SXp")
            SYp = statep.tile([128, 32, W], bf16, name="SYp")
            SPerc = statep.tile([128, 32, W], bf16, name="SPerc")
            XY = [statep.tile([128, 32, W], bf16, name=f"XY{k}") for k in range(2)]
            T1 = convp.tile([128, SR, WP], bf16, name="T1")
            T2 = convp.tile([128, SR, WP], bf16, name="T2")
            T3 = convp.tile([128, SR, WP], bf16, name="T3")
            T4 = convp.tile([128, SR, WP], bf16, name="T4")

            w1sT = wtsp.tile([128, nst, 2 * HIDDEN], bf16, name="w1sT")
            w1xy = wtsp.tile([128, 2, 2 * HIDDEN], bf16, name="w1xy")
            w2dr = wtsp.tile([HIDDEN, 2, HIDDEN], fp8, name="w2dr")
            w2drb = wtsp.tile([HIDDEN, 2, HIDDEN], fp8e5, name="w2drb")
            w3T = wtsp.tile([HIDDEN, CB], bf16, name="w3T")

            nc.sync.dma_start(out=S[0][:], in_=s0_d[:])
            nc.sync.dma_start(out=S[1][:], in_=s0_d[:])
            # seed the bf16 shadow once: outer halo rows and pad columns
            # keep these values forever (per-step copies only touch the
            # interior rows and the inner halo rows)
            nc.gpsimd.tensor_copy(out=s16[:], in_=S[0][:])
            nc.sync.dma_start(out=w1sT[:], in_=w1sT_d[:])
            nc.sync.dma_start(out=w1xy[:], in_=w1xy_d[:])
            nc.sync.dma_start(out=w2dr[:], in_=w2dr_d[:])
            nc.sync.dma_start(out=w2drb[:], in_=w2drb_d[:])
            nc.sync.dma_start(out=w3T[:], in_=w3T_d[:])

            fts = {}

            def get_ft(g):
                if g not in fts:
                    ftt = firep.tile([128, SR, WP], bf16, name="ft")
                    nc.sync.dma_start(out=ftt[:], in_=fire_d[g % nst])
                    fts[g] = ftt
                return fts[g]

            # warm the PE p-state during the init DMAs so step 0's matmuls
            # run at full clock
            for _w in range(24):
                wp_ = ph1.tile([HIDDEN, 2 * NPIX], f32, name="h1")
                nc.tensor.matmul(wp_[:, 0:HIDDEN], w1xy[:, 0, 0:HIDDEN],
                                 w1xy[:, 0, 0:HIDDEN], start=True, stop=True)

            def emit_conv_slabs(g, slabs):
                """Sobel + fired-gather ops for step g restricted to the
                given interior row-block windows.  `slabs` is a list of
                (rb_lo, rb_hi) inclusive rb ranges processed as one window
                (rows 8*rb_lo+1 .. 8*rb_hi+8)."""
                ft = get_ft(g)
                for (lo, hi) in slabs:
                    ra = 8 * lo + 1          # first interior row
                    rb_ = 8 * hi + 9         # one past last interior row
                    # sobel-x: vertical 1-2-1 (two pair-adds) then col diff
                    nc.vector.tensor_add(out=T1[:, ra - 1:rb_, :],
                                         in0=s16[:, ra - 1:rb_, :],
                                         in1=s16[:, ra:rb_ + 1, :])
                    nc.vector.tensor_add(out=T2[:, ra:rb_, :],
                                         in0=T1[:, ra - 1:rb_ - 1, :],
                                         in1=T1[:, ra:rb_, :])
                    nc.vector.tensor_sub(out=SXp[:, ra - 1:rb_ - 1, :],
                                         in0=T2[:, ra:rb_, 3:3 + W],
                                         in1=T2[:, ra:rb_, 1:1 + W])
                    # sobel-y: horizontal 1-2-1 (two pair-adds) then row diff
                    nc.vector.tensor_add(out=T3[:, ra - 1:rb_ + 1, 0:WP - 1],
                                         in0=s16[:, ra - 1:rb_ + 1, 0:WP - 1],
                                         in1=s16[:, ra - 1:rb_ + 1, 1:WP])
                    nc.vector.tensor_add(out=T4[:, ra - 1:rb_ + 1, 1:WP - 1],
                                         in0=T3[:, ra - 1:rb_ + 1, 0:WP - 2],
                                         in1=T3[:, ra - 1:rb_ + 1, 1:WP - 1])
                    nc.vector.tensor_sub(out=SYp[:, ra - 1:rb_ - 1, :],
                                         in0=T4[:, ra + 1:rb_ + 1, CL:CL + W],
                                         in1=T4[:, ra - 1:rb_ - 1, CL:CL + W])
                    # fired interior gathers (fire folds into L1; the
                    # constant-1.0 row 32b+29 turns into the fire-gated bias)
                    fi = ft[:, ra:rb_, CL:CL + W]
                    nc.vector.tensor_mul(out=SPerc[:, ra - 1:rb_ - 1, :],
                                         in0=s16[:, ra:rb_, CL:CL + W], in1=fi)
                    for i in range(2):
                        p = slice(64 * i, 64 * i + 64)
                        nc.vector.tensor_mul(out=XY[i][0:64, ra - 1:rb_ - 1, :],
                                             in0=SXp[p, ra - 1:rb_ - 1, :],
                                             in1=ft[p, ra:rb_, CL:CL + W])
                        nc.vector.tensor_mul(out=XY[i][64:128, ra - 1:rb_ - 1, :],
                                             in0=SYp[p, ra - 1:rb_ - 1, :],
                                             in1=ft[p, ra:rb_, CL:CL + W])

            def emit_tile(g, rb, b, eidx):
                t = g % nst
                p0 = CB * b
                h1 = ph1.tile([HIDDEN, 2 * NPIX], f32, name="h1")
                rhs_s = SPerc[p0:p0 + NCH + 1, rb * RPT:(rb + 1) * RPT, :]
                rhs_xy = XY[b // 2][:, rb * RPT:(rb + 1) * RPT, :]
                for hf in range(2):
                    c0 = hf * HIDDEN
                    o = h1[:, hf * NPIX:(hf + 1) * NPIX]
                    nc.tensor.matmul(o, w1sT[p0:p0 + NCH + 1, t, c0:c0 + HIDDEN],
                                     rhs_s, start=True, stop=False,
                                     tile_position=(p0, 0))
                    nc.tensor.matmul(o, w1xy[:, b % 2, c0:c0 + HIDDEN],
                                     rhs_xy, start=False, stop=True)

                h1s = actp.tile([HIDDEN, 2, NPIX], fp8, name="h1s")
                nc.scalar.activation(
                    out=h1s[:, :, :].rearrange("p a b -> p (a b)"),
                    in_=h1[:], func=LR, bias=0.0, scale=1.0, alpha=0.01)

                eng = _H2_ENG[eidx]
                h2 = ph2.tile([HIDDEN, NPIX], f32, name="h2")
                nc.tensor.matmul(h2[:], w2dr[:], h1s[:], start=True,
                                 stop=True, perf_mode=DR)
                h2s = actp.tile([HIDDEN, NPIX], bf16, name="h2s")
                if eng == "A":
                    nc.scalar.activation(out=h2s[:], in_=h2[:], func=LR,
                                         bias=0.0, scale=1.0, alpha=0.01)
                else:
                    # LeakyRelu(x) = max(x, 0.01x): 0.01x from a second DR
                    # pass with 0.01-scaled weights, then one tensor_max
                    h2b = ph2.tile([HIDDEN, NPIX], f32, name="h2")
                    nc.tensor.matmul(h2b[:], w2drb[:], h1s[:], start=True,
                                     stop=True, perf_mode=DR)
                    nc.vector.tensor_max(out=h2s[:], in0=h2[:], in1=h2b[:])
                return h2s

            def emit_step(g):
                cur, nxt = S[g % 2], S[(g + 1) % 2]
                if g + 1 < steps:
                    get_ft(g + 1)
                for wi, rb in enumerate(_WAVE_ORDER):
                    r0 = 1 + rb * RPT
                    dx = pdx.tile([128, NPIX], f32, name="dx")
                    for b in range(4):
                        h2s = emit_tile(g, rb, b, 4 * wi + b)
                        p0 = CB * b
                        nc.tensor.matmul(dx[p0:p0 + CB, :], w3T[:], h2s[:],
                                         start=True, stop=True,
                                         tile_position=(0, p0))
                    # fp32 master update on DVE (GPSIMD cannot read PSUM);
                    # bf16 shadow refresh as a Pool SBUF->SBUF copy
                    nc.vector.tensor_add(
                        out=nxt[:, r0:r0 + RPT, CL:CL + W],
                        in0=cur[:, r0:r0 + RPT, CL:CL + W],
                        in1=dx[:, :].rearrange("p (a b) -> p a b", a=RPT))
                    nc.gpsimd.tensor_copy(
                        out=s16[:, r0:r0 + RPT, CL:CL + W],
                        in_=nxt[:, r0:r0 + RPT, CL:CL + W])
                    if g + 1 < steps and wi == 2:
                        # adds for rbs {0,1,2} done: next step's slab 1
                        emit_conv_slabs(g + 1, [(1, 1)])
                if g + 1 < steps:
                    # inner halo rows of the shadow, from the neighbour
                    # half-strip's freshly updated master rows
                    for i in range(2):
                        ip = 64 * i
                        nc.gpsimd.tensor_copy(out=s16[ip + CB:ip + CB + NCH, 0:1, :],
                                              in_=nxt[ip:ip + NCH, 32:33, :])
                        nc.gpsimd.tensor_copy(out=s16[ip:ip + NCH, 33:34, :],
                                              in_=nxt[ip + CB:ip + CB + NCH, 1:2, :])
                    emit_conv_slabs(g + 1, [(0, 0), (2, 2), (3, 3)])

            emit_conv_slabs(0, [(1, 1), (0, 0), (2, 2), (3, 3)])
            for g in range(steps):
                emit_step(g)

            nc.sync.dma_start(out=out_d[:], in_=S[steps % 2][:])

    orig = nc.to_json_bytes
    nc.to_json_bytes = lambda: _fix_bir_waits(orig())
    return nc


_CACHE = {}


def _get_nc(steps: int):
    if steps not in _CACHE:
        _CACHE[steps] = _build(steps)
    return _CACHE[steps]


def _prep_inputs(x, w1, b1, w2, w3, steps):
    """Host-side input preparation; returns per-core input maps."""
    x = np.asarray(x, np.float32)
    w1 = np.asarray(w1, np.float32)
    b1 = np.asarray(b1, np.float32)
    w2 = np.asarray(w2, np.float32)
    w3 = np.asarray(w3, np.float32)

    hid, fires = _host_rng(steps)
    nst = max(steps, 1)
    bf = ml_dtypes.bfloat16
    f8 = ml_dtypes.float8_e4m3fn

    # full padded state [B, 32, 66, 68]; channel row 29 held at 1.0 (bias row)
    state0 = np.zeros((B, CB, H + 2, WP), np.float32)
    state0[:, :NUM_IMG, 1:1 + H, CL:CL + W] = x
    state0[:, NUM_IMG:NUM_IMG + NUM_HID, 1:1 + H, CL:CL + W] = hid
    state0[:, NCH, :, :] = 1.0

    # w1 split: s-tap (+ per-step bias row), sobel-x, sobel-y (sobel /8)
    w1sT = np.zeros((128, nst, 2 * HIDDEN), np.float32)
    for b0 in (0, 32, 64, 96):
        w1sT[b0:b0 + NCH] = w1[:, 0:NCH].T[:, None, :]
        for t in range(steps):
            w1sT[b0 + NCH, t] = b1 + w1[:, 3 * NCH] * (np.float32(t) / 100.0)
    w1sT = w1sT.astype(bf)

    w1xy = np.zeros((128, 2, 2 * HIDDEN), np.float32)
    for par in range(2):  # block parity: even blocks at rows 0/64, odd at 32/96
        w1xy[32 * par:32 * par + NCH, par] = w1[:, NCH:2 * NCH].T / 8.0
        w1xy[64 + 32 * par:64 + 32 * par + NCH, par] = w1[:, 2 * NCH:3 * NCH].T / 8.0
    w1xy = w1xy.astype(bf)

    w2dr = np.zeros((HIDDEN, 2, HIDDEN), np.float32)
    w2dr[:, 0, :] = w2[:, 0:HIDDEN].T
    w2dr[:, 1, :] = w2[:, HIDDEN:2 * HIDDEN].T
    w2drb = (0.01 * w2dr).astype(ml_dtypes.float8_e5m2)
    w2dr = w2dr.astype(f8)

    w3T = np.zeros((HIDDEN, CB), np.float32)
    w3T[:, :NCH] = w3.T
    w3T[:, :NUM_IMG] = 0.0       # image channels are immutable
    w3T = w3T.astype(bf)

    # fire in strip layout [steps, 128, 34, 68], per-block-matched partitions
    firest = np.zeros((nst, 128, SR, WP), np.float32)
    in_maps = []
    for c in range(N_CORES):
        imgs = slice(c * B_LOC, (c + 1) * B_LOC)
        sc = state0[imgs]                      # [B_LOC, 32, 66, 68]
        s0 = np.stack([sc[:, :, 0:SR, :], sc[:, :, 32:32 + SR, :]], axis=1)
        s0 = s0.reshape(B_LOC * 2 * CB, SR, WP)
        f = fires[:nst, imgs]                  # [steps, B_LOC, H, W]
        fs = firest.copy()
        for i in range(B_LOC):
            for hf in range(2):
                p0 = 64 * i + 32 * hf
                fs[:, p0:p0 + CB, 1:33, CL:CL + W] = \
                    f[:, i, 32 * hf:32 * hf + 32, :][:, None]
        in_maps.append({
            "s0": np.ascontiguousarray(s0),
            "fire": np.ascontiguousarray(fs).astype(bf),
            "w1sT": w1sT, "w1xy": w1xy, "w2dr": w2dr, "w2drb": w2drb,
            "w3T": w3T,
        })
    return in_maps


def _softmax(x):
    m = x.max(axis=-1, keepdims=True)
    e = np.exp(x - m)
    return e / e.sum(axis=-1, keepdims=True)


def _epilogue(results):
    logits = np.zeros((B, NUM_OUT), np.float32)
    for c, res in enumerate(results):
        so = res["sout"].reshape(B_LOC, 2, CB, SR, WP)
        cls = so[:, :, NUM_IMG + NUM_HID:NCH, 1:33, CL:CL + W]
        logits[c * B_LOC:(c + 1) * B_LOC] = cls.mean(axis=(1, 3, 4))
    return _softmax(logits).astype(np.float32)


def _run(trace=False, _in_maps=None, **inputs):
    from concourse.bass_utils import run_bass_kernel_spmd
    steps = int(inputs["steps"])
    if steps == 0:
        return _softmax(np.zeros((B, NUM_OUT), np.float32)), None
    in_maps = _in_maps
    if in_maps is None:
        in_maps = _prep_inputs(inputs["x"], inputs["w1"], inputs["b1"],
                               inputs["w2"], inputs["w3"], steps)
    nc = _get_nc(steps)
    r = run_bass_kernel_spmd(nc, in_maps, core_ids=list(range(N_CORES)),
                             trace=trace)
    return _epilogue(r.results), r.exec_time_ns


def predicted_exec_ns(steps: int = 20) -> float:
    """Cost-model (TimelineSim) estimate of on-device execution time for the
    whole job (all cores run the same program in parallel)."""
    from concourse.timeline_sim import TimelineSim
    nc = _build(int(steps))
    return TimelineSim(nc, trace=False).simulate()


def kernel(**inputs) -> np.ndarray:
    out, _ = _run(trace=False, **inputs)
    return out


# revision 18
# speedup vs baseline: 1.2986x; 1.0024x over previous
"""Trainium2 Bass kernel for the ClassificationNCA problem.

Self-contained: callable as kernel(**inputs) with the full (unsharded)
inputs; shards batch across 8 NeuronCores (2 images/core), runs the
20-step NCA entirely in SBUF, returns softmax(mean-pooled class channels).

Layout (per core): state lives in SBUF as [128, 34, 68] fp32 where
partition = 32*block + ch, block = 2*img + half (29 channels + pad;
partition 32b+29 is held at constant 1.0 and acts as the bias input row).
Each block holds a 32-row half-image strip with halo rows at the strip
edges and columns zero-padded to 68.

Per step: full-width (128-partition) separable sobel in bf16; the fire
mask is folded into the L1 matmul inputs (fired shadow SPerc / fired
sobel gathers XY, with the constant-1.0 row turning into the per-pixel
fire value that gates the folded bias), so nothing downstream of L1
needs a fire multiply; L2 runs as an fp8e4 DoubleRow matmul; state
updates accumulate 4 blocks at once from a shared PSUM dx tile.
"""
import sys

sys.path.insert(0, "/opt/trn_rl_repo")

import json
import numpy as np
import ml_dtypes

NUM_IMG, NUM_HID, NUM_OUT = 3, 16, 10
NCH = 29            # total channels
HIDDEN = 128
B, H, W = 16, 64, 64
N_CORES = 8
B_LOC = B // N_CORES          # images per core
WP = W + 4                    # padded width: 2 pad cols each side
SR = 34                       # strip rows: pad/halo + 32 interior + halo/pad
SFLAT = SR * WP               # 2312
RPT = 8                       # rows per matmul tile
N_RB = 32 // RPT              # 4 row blocks per strip
NPIX = RPT * W                # 512 pixels per tile
CB = 32                       # partition stride per block
CL = 2                        # interior column offset

_MAX_WAITS = 1

# h2 leaky-relu engine per tile emission index: balance Act/Pool/DVE.
# "A": Activation-engine LeakyRelu; "V"/"P": dual-matmul trick -- a second
# DoubleRow pass with 0.01-scaled weights, then tensor_max on DVE/Pool.
_H2_ENG = ["A"] * 16
# wave processes row-blocks in this order so the next step's conv slabs
# (chain {1,2}, then slab 0, slab 3) become ready in consumption order
_WAVE_ORDER = [1, 0, 2, 3]


def _fix_bir_waits(bir_bytes: bytes) -> bytes:
    """walrus codegen allows only one embedded sem-wait per instruction;
    Tile sometimes attaches more (e.g. the kernel-tail drain).  Move excess
    waits onto NoOp carrier instructions inserted before the offender on the
    same engine."""
    bir = json.loads(bir_bytes)
    uid = 0
    for fn in bir["functions"]:
        for blk in fn["blocks"]:
            out = []
            for ins in blk["instructions"]:
                si = ins.get("sync_info")
                waits = (si or {}).get("on_wait") or []
                if len(waits) > _MAX_WAITS:
                    excess = waits[:-_MAX_WAITS]
                    si["on_wait"] = waits[-_MAX_WAITS:]
                    for i in range(0, len(excess), _MAX_WAITS):
                        out.append({
                            "opcode": "NoOp",
                            "name": f"wsplit_{uid}",
                            "engine": ins["engine"],
                            "ins": [],
                            "outs": [],
                            "sync_info": {
                                "on_wait": excess[i:i + _MAX_WAITS],
                                "on_update": [],
                            },
                        })
                        uid += 1
                out.append(ins)
            blk["instructions"] = out
    return json.dumps(bir).encode()


def _host_rng(steps: int):
    """Reproduce the reference's jax threefry random draws exactly (on CPU)."""
    import jax
    cpu = jax.devices("cpu")[0]
    with jax.default_device(cpu):
        import jax.numpy as jnp
        base = jax.random.key(42)
        hid = 0.5 + 0.225 * jax.random.normal(
            jax.random.fold_in(base, 10_000), (B, NUM_HID, H, W),
            dtype=jnp.float32)
        hid = np.asarray(hid)
        fires = np.zeros((max(steps, 1), B, H, W), np.float32)
        for s in range(steps):
            u = jax.random.uniform(jax.random.fold_in(base, s), (B, H, W, 1),
                                   dtype=jnp.float32)
            fires[s] = np.asarray(u < 0.5, np.float32)[..., 0]
    return hid, fires


def _build(steps: int):
    from concourse import mybir
    from concourse.bass import Bass
    from concourse.tile import TileContext

    f32 = mybir.dt.float32
    bf16 = mybir.dt.bfloat16
    fp8 = mybir.dt.float8e4
    fp8e5 = mybir.dt.float8e5
    LR = mybir.ActivationFunctionType.Lrelu
    DR = mybir.MatmulPerfMode.DoubleRow
    MULT = mybir.AluOpType.mult
    MAX = mybir.AluOpType.max

    nst = max(steps, 1)

    nc = Bass(trn_type="TRN2", target_bir_lowering=False)

    s0_d = nc.dram_tensor("s0", [128, SR, WP], f32, kind="ExternalInput")
    fire_d = nc.dram_tensor("fire", [nst, 128, SR, WP], bf16,
                            kind="ExternalInput")
    w1sT_d = nc.dram_tensor("w1sT", [128, nst, 2 * HIDDEN], bf16,
                            kind="ExternalInput")
    w1xy_d = nc.dram_tensor("w1xy", [128, 2, 2 * HIDDEN], bf16,
                            kind="ExternalInput")
    w2dr_d = nc.dram_tensor("w2dr", [HIDDEN, 2, HIDDEN], fp8, kind="ExternalInput")
    w2drb_d = nc.dram_tensor("w2drb", [HIDDEN, 2, HIDDEN], fp8e5, kind="ExternalInput")
    w3T_d = nc.dram_tensor("w3T", [HIDDEN, CB], bf16, kind="ExternalInput")
    out_d = nc.dram_tensor("sout", [128, SR, WP], f32, kind="ExternalOutput")

    with TileContext(nc) as tc:
        with tc.tile_pool(name="state", bufs=1) as statep, \
             tc.tile_pool(name="wts", bufs=1) as wtsp, \
             tc.tile_pool(name="convs", bufs=1) as convp, \
             tc.tile_pool(name="fire", bufs=2) as firep, \
             tc.tile_pool(name="acts", bufs=4) as actp, \
             tc.tile_pool(name="ph1", bufs=2, space="PSUM") as ph1, \
             tc.tile_pool(name="ph2", bufs=1, space="PSUM") as ph2, \
             tc.tile_pool(name="pdx", bufs=2, space="PSUM") as pdx:

            S = [statep.tile([128, SR, WP], f32, name=f"S{k}") for k in range(2)]
            s16 = statep.tile([128, SR, WP], bf16, name="s16")
            SXp = statep.tile([128, 32, W], bf16, name="SXp")
            SYp = statep.tile([128, 32, W], bf16, name="SYp")
            SPerc = statep.tile([128, 32, W], bf16, name="SPerc")
            XY = [statep.tile([128, 32, W], bf16, name=f"XY{k}") for k in range(2)]
            T1 = convp.tile([128, SR, WP], bf16, name="T1")
            T2 = convp.tile([128, SR, WP], bf16, name="T2")
            T3 = convp.tile([128, SR, WP], bf16, name="T3")
            T4 = convp.tile([128, SR, WP], bf16, name="T4")

            w1sT = wtsp.tile([128, nst, 2 * HIDDEN], bf16, name="w1sT")
            w1xy = wtsp.tile([128, 2, 2 * HIDDEN], bf16, name="w1xy")
            w2dr = wtsp.tile([HIDDEN, 2, HIDDEN], fp8, name="w2dr")
            w2drb = wtsp.tile([HIDDEN, 2, HIDDEN], fp8e5, name="w2drb")
            w3T = wtsp.tile([HIDDEN, CB], bf16, name="w3T")

            nc.sync.dma_start(out=S[0][:], in_=s0_d[:])
            nc.sync.dma_start(out=S[1][:], in_=s0_d[:])
            # seed the bf16 shadow once: outer halo rows and pad columns
            # keep these values forever (per-step copies only touch the
            # interior rows and the inner halo rows)
            nc.gpsimd.tensor_copy(out=s16[:], in_=S[0][:])
            nc.sync.dma_start(out=w1sT[:], in_=w1sT_d[:])
            nc.sync.dma_start(out=w1xy[:], in_=w1xy_d[:])
            nc.sync.dma_start(out=w2dr[:], in_=w2dr_d[:])
            nc.sync.dma_start(out=w2drb[:], in_=w2drb_d[:])
            nc.sync.dma_start(out=w3T[:], in_=w3T_d[:])

            fts = {}

            def get_ft(g):
                if g not in fts:
                    ftt = firep.tile([128, SR, WP], bf16, name="ft")
                    nc.sync.dma_start(out=ftt[:], in_=fire_d[g % nst])
                    fts[g] = ftt
                return fts[g]

            # warm the PE p-state during the init DMAs so step 0's matmuls
            # run at full clock
            for _w in range(24):
                wp_ = ph1.tile([HIDDEN, 2 * NPIX], f32, name="h1")
                nc.tensor.matmul(wp_[:, 0:HIDDEN], w1xy[:, 0, 0:HIDDEN],
                                 w1xy[:, 0, 0:HIDDEN], start=True, stop=True)

            def emit_conv_slabs(g, slabs):
                """Sobel + fired-gather ops for step g restricted to the
                given interior row-block windows.  `slabs` is a list of
                (rb_lo, rb_hi) inclusive rb ranges processed as one window
                (rows 8*rb_lo+1 .. 8*rb_hi+8)."""
                ft = get_ft(g)
                for (lo, hi) in slabs:
                    ra = 8 * lo + 1          # first interior row
                    rb_ = 8 * hi + 9         # one past last interior row
                    # sobel-x: vertical 1-2-1 (two pair-adds) then col diff
                    nc.vector.tensor_add(out=T1[:, ra - 1:rb_, :],
                                         in0=s16[:, ra - 1:rb_, :],
                                         in1=s16[:, ra:rb_ + 1, :])
                    nc.vector.tensor_add(out=T2[:, ra:rb_, :],
                                         in0=T1[:, ra - 1:rb_ - 1, :],
                                         in1=T1[:, ra:rb_, :])
                    nc.vector.tensor_sub(out=SXp[:, ra - 1:rb_ - 1, :],
                                         in0=T2[:, ra:rb_, 3:3 + W],
                                         in1=T2[:, ra:rb_, 1:1 + W])
                    # sobel-y: horizontal 1-2-1 (two pair-adds) then row diff
                    nc.vector.tensor_add(out=T3[:, ra - 1:rb_ + 1, 0:WP - 1],
                                         in0=s16[:, ra - 1:rb_ + 1, 0:WP - 1],
                                         in1=s16[:, ra - 1:rb_ + 1, 1:WP])
                    nc.vector.tensor_add(out=T4[:, ra - 1:rb_ + 1, 1:WP - 1],
                                         in0=T3[:, ra - 1:rb_ + 1, 0:WP - 2],
                                         in1=T3[:, ra - 1:rb_ + 1, 1:WP - 1])
                    nc.vector.tensor_sub(out=SYp[:, ra - 1:rb_ - 1, :],
                                         in0=T4[:, ra + 1:rb_ + 1, CL:CL + W],
                                         in1=T4[:, ra - 1:rb_ - 1, CL:CL + W])
                    # fired interior gathers (fire folds into L1; the
                    # constant-1.0 row 32b+29 turns into the fire-gated bias)
                    fi = ft[:, ra:rb_, CL:CL + W]
                    nc.vector.tensor_mul(out=SPerc[:, ra - 1:rb_ - 1, :],
                                         in0=s16[:, ra:rb_, CL:CL + W], in1=fi)
                    for i in range(2):
                        p = slice(64 * i, 64 * i + 64)
                        nc.vector.tensor_mul(out=XY[i][0:64, ra - 1:rb_ - 1, :],
                                             in0=SXp[p, ra - 1:rb_ - 1, :],
                                             in1=ft[p, ra:rb_, CL:CL + W])
                        nc.vector.tensor_mul(out=XY[i][64:128, ra - 1:rb_ - 1, :],
                                             in0=SYp[p, ra - 1:rb_ - 1, :],
                                             in1=ft[p, ra:rb_, CL:CL + W])

            def emit_l1(g, rb, b):
                t = g % nst
                p0 = CB * b
                h1 = ph1.tile([HIDDEN, 2 * NPIX], f32, name="h1")
                rhs_s = SPerc[p0:p0 + NCH + 1, rb * RPT:(rb + 1) * RPT, :]
                rhs_xy = XY[b // 2][:, rb * RPT:(rb + 1) * RPT, :]
                for hf in range(2):
                    c0 = hf * HIDDEN
                    o = h1[:, hf * NPIX:(hf + 1) * NPIX]
                    nc.tensor.matmul(o, w1sT[p0:p0 + NCH + 1, t, c0:c0 + HIDDEN],
                                     rhs_s, start=True, stop=False,
                                     tile_position=(p0, 0))
                    nc.tensor.matmul(o, w1xy[:, b % 2, c0:c0 + HIDDEN],
                                     rhs_xy, start=False, stop=True)
                h1s = actp.tile([HIDDEN, 2, NPIX], fp8, name="h1s")
                nc.scalar.activation(
                    out=h1s[:, :, :].rearrange("p a b -> p (a b)"),
                    in_=h1[:], func=LR, bias=0.0, scale=1.0, alpha=0.01)
                return h1s

            def emit_pair(g, rb, bp):
                """Two tiles (b = 2*bp, 2*bp+1) share one h2 PSUM pair tile
                and one [128,1024] h2 activation."""
                h1s0 = emit_l1(g, rb, 2 * bp)
                h1s1 = emit_l1(g, rb, 2 * bp + 1)
                h2 = ph2.tile([HIDDEN, 2 * NPIX], f32, name="h2")
                nc.tensor.matmul(h2[:, 0:NPIX], w2dr[:], h1s0[:], start=True,
                                 stop=True, perf_mode=DR)
                nc.tensor.matmul(h2[:, NPIX:2 * NPIX], w2dr[:], h1s1[:],
                                 start=True, stop=True, perf_mode=DR)
                h2s = actp.tile([HIDDEN, 2 * NPIX], bf16, name="h2s")
                nc.scalar.activation(out=h2s[:], in_=h2[:], func=LR,
                                     bias=0.0, scale=1.0, alpha=0.01)
                return h2s

            def emit_step(g):
                cur, nxt = S[g % 2], S[(g + 1) % 2]
                if g + 1 < steps:
                    get_ft(g + 1)
                for wi, rb in enumerate(_WAVE_ORDER):
                    r0 = 1 + rb * RPT
                    dx = pdx.tile([128, NPIX], f32, name="dx")
                    for bp in range(2):
                        h2s = emit_pair(g, rb, bp)
                        for j in range(2):
                            p0 = CB * (2 * bp + j)
                            nc.tensor.matmul(dx[p0:p0 + CB, :], w3T[:],
                                             h2s[:, j * NPIX:(j + 1) * NPIX],
                                             start=True, stop=True,
                                             tile_position=(0, p0))
                    # fp32 master update on DVE (GPSIMD cannot read PSUM);
                    # bf16 shadow refresh as a Pool SBUF->SBUF copy
                    nc.vector.tensor_add(
                        out=nxt[:, r0:r0 + RPT, CL:CL + W],
                        in0=cur[:, r0:r0 + RPT, CL:CL + W],
                        in1=dx[:, :].rearrange("p (a b) -> p a b", a=RPT))
                    nc.gpsimd.tensor_copy(
                        out=s16[:, r0:r0 + RPT, CL:CL + W],
                        in_=nxt[:, r0:r0 + RPT, CL:CL + W])
                    if g + 1 < steps and wi == 2:
                        # adds for rbs {0,1,2} done: next step's slab 1
                        emit_conv_slabs(g + 1, [(1, 1)])
                if g + 1 < steps:
                    # inner halo rows of the shadow, from the neighbour
                    # half-strip's freshly updated master rows
                    for i in range(2):
                        ip = 64 * i
                        nc.gpsimd.tensor_copy(out=s16[ip + CB:ip + CB + NCH, 0:1, :],
                                              in_=nxt[ip:ip + NCH, 32:33, :])
                        nc.gpsimd.tensor_copy(out=s16[ip:ip + NCH, 33:34, :],
                                              in_=nxt[ip + CB:ip + CB + NCH, 1:2, :])
                    emit_conv_slabs(g + 1, [(0, 0), (2, 2), (3, 3)])

            emit_conv_slabs(0, [(1, 1), (0, 0), (2, 2), (3, 3)])
            for g in range(steps):
                emit_step(g)

            nc.sync.dma_start(out=out_d[:], in_=S[steps % 2][:])

    orig = nc.to_json_bytes
    nc.to_json_bytes = lambda: _fix_bir_waits(orig())
    return nc


_CACHE = {}


def _get_nc(steps: int):
    if steps not in _CACHE:
        _CACHE[steps] = _build(steps)
    return _CACHE[steps]


def _prep_inputs(x, w1, b1, w2, w3, steps):
    """Host-side input preparation; returns per-core input maps."""
    x = np.asarray(x, np.float32)
    w1 = np.asarray(w1, np.float32)
    b1 = np.asarray(b1, np.float32)
    w2 = np.asarray(w2, np.float32)
    w3 = np.asarray(w3, np.float32)

    hid, fires = _host_rng(steps)
    nst = max(steps, 1)
    bf = ml_dtypes.bfloat16
    f8 = ml_dtypes.float8_e4m3fn

    # full padded state [B, 32, 66, 68]; channel row 29 held at 1.0 (bias row)
    state0 = np.zeros((B, CB, H + 2, WP), np.float32)
    state0[:, :NUM_IMG, 1:1 + H, CL:CL + W] = x
    state0[:, NUM_IMG:NUM_IMG + NUM_HID, 1:1 + H, CL:CL + W] = hid
    state0[:, NCH, :, :] = 1.0

    # w1 split: s-tap (+ per-step bias row), sobel-x, sobel-y (sobel /8)
    w1sT = np.zeros((128, nst, 2 * HIDDEN), np.float32)
    for b0 in (0, 32, 64, 96):
        w1sT[b0:b0 + NCH] = w1[:, 0:NCH].T[:, None, :]
        for t in range(steps):
            w1sT[b0 + NCH, t] = b1 + w1[:, 3 * NCH] * (np.float32(t) / 100.0)
    w1sT = w1sT.astype(bf)

    w1xy = np.zeros((128, 2, 2 * HIDDEN), np.float32)
    for par in range(2):  # block parity: even blocks at rows 0/64, odd at 32/96
        w1xy[32 * par:32 * par + NCH, par] = w1[:, NCH:2 * NCH].T / 8.0
        w1xy[64 + 32 * par:64 + 32 * par + NCH, par] = w1[:, 2 * NCH:3 * NCH].T / 8.0
    w1xy = w1xy.astype(bf)

    w2dr = np.zeros((HIDDEN, 2, HIDDEN), np.float32)
    w2dr[:, 0, :] = w2[:, 0:HIDDEN].T
    w2dr[:, 1, :] = w2[:, HIDDEN:2 * HIDDEN].T
    w2drb = (0.01 * w2dr).astype(ml_dtypes.float8_e5m2)
    w2dr = w2dr.astype(f8)

    w3T = np.zeros((HIDDEN, CB), np.float32)
    w3T[:, :NCH] = w3.T
    w3T[:, :NUM_IMG] = 0.0       # image channels are immutable
    w3T = w3T.astype(bf)

    # fire in strip layout [steps, 128, 34, 68], per-block-matched partitions
    firest = np.zeros((nst, 128, SR, WP), np.float32)
    in_maps = []
    for c in range(N_CORES):
        imgs = slice(c * B_LOC, (c + 1) * B_LOC)
        sc = state0[imgs]                      # [B_LOC, 32, 66, 68]
        s0 = np.stack([sc[:, :, 0:SR, :], sc[:, :, 32:32 + SR, :]], axis=1)
        s0 = s0.reshape(B_LOC * 2 * CB, SR, WP)
        f = fires[:nst, imgs]                  # [steps, B_LOC, H, W]
        fs = firest.copy()
        for i in range(B_LOC):
            for hf in range(2):
                p0 = 64 * i + 32 * hf
                fs[:, p0:p0 + CB, 1:33, CL:CL + W] = \
                    f[:, i, 32 * hf:32 * hf + 32, :][:, None]
        in_maps.append({
            "s0": np.ascontiguousarray(s0),
            "fire": np.ascontiguousarray(fs).astype(bf),
            "w1sT": w1sT, "w1xy": w1xy, "w2dr": w2dr, "w2drb": w2drb,
            "w3T": w3T,
        })
    return in_maps


def _softmax(x):
    m = x.max(axis=-1, keepdims=True)
    e = np.exp(x - m)
    return e / e.sum(axis=-1, keepdims=True)


def _epilogue(results):
    logits = np.zeros((B, NUM_OUT), np.float32)
    for c, res in enumerate(results):
        so = res["sout"].reshape(B_LOC, 2, CB, SR, WP)
        cls = so[:, :, NUM_IMG + NUM_HID:NCH, 1:33, CL:CL + W]
        logits[c * B_LOC:(c + 1) * B_LOC] = cls.mean(axis=(1, 3, 4))
    return _softmax(logits).astype(np.float32)


def _run(trace=False, _in_maps=None, **inputs):
    from concourse.bass_utils import run_bass_kernel_spmd
    steps = int(inputs["steps"])
    if steps == 0:
        return _softmax(np.zeros((B, NUM_OUT), np.float32)), None
    in_maps = _in_maps
    if in_maps is None:
        in_maps = _prep_inputs(inputs["x"], inputs["w1"], inputs["b1"],
                               inputs["w2"], inputs["w3"], steps)
    nc = _get_nc(steps)
    r = run_bass_kernel_spmd(nc, in_maps, core_ids=list(range(N_CORES)),
                             trace=trace)
    return _epilogue(r.results), r.exec_time_ns


def predicted_exec_ns(steps: int = 20) -> float:
    """Cost-model (TimelineSim) estimate of on-device execution time for the
    whole job (all cores run the same program in parallel)."""
    from concourse.timeline_sim import TimelineSim
    nc = _build(int(steps))
    return TimelineSim(nc, trace=False).simulate()


def kernel(**inputs) -> np.ndarray:
    out, _ = _run(trace=False, **inputs)
    return out
